# revision 1
# baseline (speedup 1.0000x reference)
"""Trainium2 Bass kernel for nn_CPAMDec_Mix (dual cross-attention, CPAM decoder).

Math (per batch element n):
    q_i = (wq_i @ x_i + bq_i)            # (D, HW)   1x1 conv query
    k_i = y_i @ wk_i.T + bk_i            # (K, D)    linear key
    v_i = y_i @ wv_i.T + bv_i            # (K, C)    linear value
    e   = | q_1.T k_1.T - q_2.T k_2.T |  # (HW, K)
    a   = softmax_K(e)
    out_i = scale * (v_i.T @ a.T) + x_i  # (C, HW)

Sharding: pure data parallel, one batch element per NeuronCore (N=8, 8 cores).
All weights replicated.  Host-side marshaling pre-transposes the small weight
matrices / y tensors so the contraction dim (C) lands on SBUF partitions.

On-chip layout per core (everything streamed over pixel tiles of L=512):
    E^T (K x L) layout keeps softmax results directly usable as the moving
    operand of the output matmul (contract over K).  Softmax over K (the
    partition dim) is done with ones-matmuls: S = 1.T @ exp(E), then
    R = 1/S broadcast back over K partitions with another ones-matmul.
    exp() needs no max-subtraction: energies are |.| >= 0 and bounded
    (~20 for this operator scale), far from fp32 overflow.
    Matmuls run as float32r (fp32 bits, replicated fast path: 1 PE
    cycle/row for moving >= 256 instead of 4 for plain fp32).  The BIR
    verifier requires every f32r matmul operand to be produced as f32r,
    so matmul-feeding DRAM tensors/tiles are declared f32r end-to-end;
    the residual add reads the x tiles bitcast back to f32 (exact bits).
"""

import numpy as np

N, C, H, W, K = 8, 512, 64, 64, 64
HW = H * W          # 4096
D = C // 4          # 128
L = 512             # pixel tile size
NT = HW // L        # 8 tiles
NCH = C // 128      # 4 contraction chunks
P = 128

_CACHE = {}


def _build():
    from contextlib import ExitStack

    import concourse.tile as tile
    from concourse import bacc, mybir

    f32 = mybir.dt.float32
    f32r = mybir.dt.float32r
    bf16 = mybir.dt.bfloat16
    AF = mybir.ActivationFunctionType
    ALU = mybir.AluOpType

    nc = bacc.Bacc("TRN2", target_bir_lowering=False, debug=False)

    def din(name, shape, dt=f32):
        return nc.dram_tensor(name, shape, dt, kind="ExternalInput").ap()

    def dout(name, shape):
        return nc.dram_tensor(name, shape, f32, kind="ExternalOutput").ap()

    x1 = din("x1", [C, HW], f32r)
    x2 = din("x2", [C, HW], f32r)
    # k/v-side tensors come in as bf16 (they feed the bf16 E/U path)
    y1t = din("y1t", [C, K], bf16)
    y2t = din("y2t", [C, K], bf16)
    wq1t = din("wq1t", [C, D], f32r)
    wq2t = din("wq2t", [C, D], f32r)
    wk1t = din("wk1t", [C, D], bf16)
    wk2t = din("wk2t", [C, D], bf16)
    wv1t = din("wv1t", [C, C], bf16)
    wv2t = din("wv2t", [C, C], bf16)
    bq1 = din("bq1", [D, 1])
    bq2 = din("bq2", [D, 1])
    bk1 = din("bk1", [D, 1])
    bk2 = din("bk2", [D, 1])
    bv1 = din("bv1", [1, C], bf16)
    bv2 = din("bv2", [1, C], bf16)
    ones_r = din("ones_r", [1, K], bf16)
    ones_c = din("ones_c", [K, 1], bf16)
    scol = din("scol", [P, 1])  # scale broadcast to 128 partitions (host)
    o1 = dout("o1", [C, HW])
    o2 = dout("o2", [C, HW])

    # chunked (partition-major) views of the DRAM tensors
    x1r = x1.rearrange("(c p) l -> c p l", p=P)
    x2r = x2.rearrange("(c p) l -> c p l", p=P)
    o1r = o1.rearrange("(c p) l -> c p l", p=P)
    o2r = o2.rearrange("(c p) l -> c p l", p=P)
    y1r = y1t.rearrange("(c p) k -> c p k", p=P)
    y2r = y2t.rearrange("(c p) k -> c p k", p=P)
    wq1r = wq1t.rearrange("(c p) d -> c p d", p=P)
    wq2r = wq2t.rearrange("(c p) d -> c p d", p=P)
    wk1r = wk1t.rearrange("(c p) d -> c p d", p=P)
    wk2r = wk2t.rearrange("(c p) d -> c p d", p=P)
    wv1r = wv1t.rearrange("(c p) e -> c p e", p=P)
    wv2r = wv2t.rearrange("(c p) e -> c p e", p=P)

    with tile.TileContext(nc) as tc, ExitStack() as ctx:
        cpool = ctx.enter_context(tc.tile_pool(name="const", bufs=1))

        # --- load replicated constants -------------------------------------
        def load_chunks(name, src_r, nchunks, width, dt=f32r, eng=None):
            t = cpool.tile([P, nchunks * width], dt, name=name, tag=name)
            for j in range(nchunks):
                (eng or nc.sync).dma_start(
                    t[:, j * width:(j + 1) * width], src_r[j])
            return t

        # small k/q-side weights on the load (SP) ring; the big wv tensors
        # ride the otherwise-idle Activation ring so tile-0 x loads aren't
        # queued behind them
        y1s = load_chunks("y1s", y1r, NCH, K, bf16)
        y2s = load_chunks("y2s", y2r, NCH, K, bf16)
        wk1s = load_chunks("wk1s", wk1r, NCH, D, bf16)
        wk2s = load_chunks("wk2s", wk2r, NCH, D, bf16)
        wq1s = load_chunks("wq1s", wq1r, NCH, D)
        wq2s = load_chunks("wq2s", wq2r, NCH, D)
        wv1s = load_chunks("wv1s", wv1r, NCH, C, bf16, eng=nc.scalar)
        wv2s = load_chunks("wv2s", wv2r, NCH, C, bf16, eng=nc.scalar)

        def load1(name, src, shape, dt=f32):
            t = cpool.tile(shape, dt, name=name, tag=name)
            nc.sync.dma_start(t[:], src[:])
            return t

        bq1s = load1("bq1s", bq1, [D, 1])
        bq2s = load1("bq2s", bq2, [D, 1])
        bk1s = load1("bk1s", bk1, [D, 1])
        bk2s = load1("bk2s", bk2, [D, 1])
        bv1s = load1("bv1s", bv1, [1, C], bf16)
        bv2s = load1("bv2s", bv2, [1, C], bf16)
        onrs = load1("onrs", ones_r, [1, K], bf16)
        oncs = load1("oncs", ones_c, [K, 1], bf16)
        scols = load1("scols", scol, [P, 1])

        bk2n = cpool.tile([D, 1], f32, name="bk2n", tag="bk2n")
        nc.scalar.mul(bk2n[:], bk2s[:], -1.0)

        # --- setup: K1t (D,K), K2tn = -(K2t+bk2), V1 (K,C), V2 (K,C) -------
        # bf16: these feed the E/U matmuls (1 cyc/row vs 2 for f32r)
        k1s = cpool.tile([D, K], bf16, name="k1s", tag="k1s")
        k2ns = cpool.tile([D, K], bf16, name="k2ns", tag="k2ns")
        v1s = cpool.tile([K, C], bf16, name="v1s", tag="v1s")
        v2s = cpool.tile([K, C], bf16, name="v2s", tag="v2s")

        with ExitStack() as sctx:
            spsum = sctx.enter_context(
                tc.tile_pool(name="spsum", bufs=1, space="PSUM"))

            for (wks, ys, ks, bias, sc) in (
                    (wk1s, y1s, k1s, bk1s, 1.0),
                    (wk2s, y2s, k2ns, bk2n, -1.0)):
                kp = spsum.tile([D, K], f32, name="kp", tag="kp")
                for j in range(NCH):
                    nc.tensor.matmul(
                        kp[:],
                        wks[:, j * D:(j + 1) * D],
                        ys[:, j * K:(j + 1) * K],
                        start=(j == 0), stop=(j == NCH - 1))
                # ks = sc*kp + bias  (sc=-1, bias=-bk2 negates K2t + bk2)
                nc.scalar.activation(ks[:], kp[:], AF.Identity,
                                     bias=bias[:], scale=sc)

            for (ys, wvs, bvs, vs) in (
                    (y1s, wv1s, bv1s, v1s), (y2s, wv2s, bv2s, v2s)):
                vp = spsum.tile([K, C], f32, name="vp", tag="vp")
                for j in range(NCH):
                    nc.tensor.matmul(
                        vp[:],
                        ys[:, j * K:(j + 1) * K],
                        wvs[:, j * C:(j + 1) * C],
                        start=(j == 0), stop=False)
                # += ones.T @ bv  (broadcast bias add over K partitions)
                nc.tensor.matmul(vp[:], onrs[:], bvs[:], start=False,
                                 stop=True)
                nc.scalar.copy(vs[:], vp[:])

        # --- streaming pools ----------------------------------------------
        xpool = ctx.enter_context(tc.tile_pool(name="xpool", bufs=4))
        qsb = ctx.enter_context(tc.tile_pool(name="qsb", bufs=3))
        softp = ctx.enter_context(tc.tile_pool(name="softp", bufs=3))
        opool = ctx.enter_context(tc.tile_pool(name="opool", bufs=3))
        qpp = ctx.enter_context(tc.tile_pool(name="qpp", bufs=1, space="PSUM"))
        epp = ctx.enter_context(tc.tile_pool(name="epp", bufs=2, space="PSUM"))
        spp = ctx.enter_context(tc.tile_pool(name="spp", bufs=1, space="PSUM"))
        upp = ctx.enter_context(tc.tile_pool(name="upp", bufs=2, space="PSUM"))

        for t in range(NT):
            l0 = t * L
            xts = {}
            for s, xr in ((0, x1r), (1, x2r)):
                # per-stream tile holding all 4 channel chunks side by side.
                # All loads go on the SP HWDGE ring, all stores on the
                # Activation ring: a ring is FIFO, so mixing loads behind
                # compute-gated stores head-of-line-blocks the loads.
                xt = xpool.tile([P, NCH * L], f32r, name=f"x{s}", tag=f"x{s}")
                for j in range(NCH):
                    nc.sync.dma_start(xt[:, j * L:(j + 1) * L],
                                      xr[j][:, l0:l0 + L])
                xts[s] = xt

            qs = []
            for s, (wqs, bqs) in enumerate(((wq1s, bq1s), (wq2s, bq2s))):
                qp = qpp.tile([D, L], f32, name=f"q{s}p", tag=f"q{s}p")
                for j in range(NCH):
                    nc.tensor.matmul(
                        qp[:],
                        wqs[:, j * D:(j + 1) * D],
                        xts[s][:, j * L:(j + 1) * L],
                        start=(j == 0), stop=(j == NCH - 1))
                q = qsb.tile([D, L], bf16, name=f"q{s}s", tag=f"q{s}s")
                nc.scalar.activation(q[:], qp[:], AF.Identity, bias=bqs[:])
                qs.append(q)

            ep = epp.tile([K, L], f32, name="ep", tag="ep")
            nc.tensor.matmul(ep[:], k1s[:], qs[0][:], start=True, stop=False)
            nc.tensor.matmul(ep[:], k2ns[:], qs[1][:], start=False, stop=True)

            aabs = softp.tile([K, L], f32, name="aabs", tag="aabs")
            nc.scalar.activation(aabs[:], ep[:], AF.Abs)
            expe = softp.tile([K, L], bf16, name="expe", tag="expe")
            nc.scalar.activation(expe[:], aabs[:], AF.Exp)

            sp = spp.tile([1, L], f32, name="sp", tag="sp")
            nc.tensor.matmul(sp[:], oncs[:], expe[:], start=True, stop=True)
            rs = softp.tile([1, L], f32, name="rs", tag="rs")
            # 1/S at ~18 bits; S in [K, K*exp(~20)] so no edge cases
            nc.vector.reciprocal_approx_fast(rs[:], sp[:])
            rsb = softp.tile([1, L], bf16, name="rsb", tag="rsb")
            nc.scalar.copy(rsb[:], rs[:])
            rbp = spp.tile([K, L], f32, name="rbp", tag="rbp")
            nc.tensor.matmul(rbp[:], onrs[:], rsb[:], start=True, stop=True)
            attn = softp.tile([K, L], bf16, name="attn", tag="attn")
            nc.vector.tensor_mul(attn[:], expe[:], rbp[:])

            for s, (vs, orr) in enumerate(((v1s, o1r), (v2s, o2r))):
                ot = opool.tile([P, NCH * L], f32, name=f"ot{s}", tag=f"ot{s}")
                for j in range(NCH):
                    up = upp.tile([P, L], f32, name="up", tag="up")
                    nc.tensor.matmul(up[:], vs[:, j * P:(j + 1) * P],
                                     attn[:], start=True, stop=True)
                    # ot = (up * scale) + x in one DVE op
                    nc.vector.scalar_tensor_tensor(
                        ot[:, j * L:(j + 1) * L], up[:], scols[:],
                        xts[s][:, j * L:(j + 1) * L].bitcast(f32),
                        ALU.mult, ALU.add)
                    # stream-0 stores ride the SWDGE (gpsimd) queues,
                    # stream-1 the Activation HWDGE ring; the SP ring
                    # stays dedicated to loads
                    steng = nc.gpsimd if s == 0 else nc.scalar
                    steng.dma_start(orr[j][:, l0:l0 + L],
                                    ot[:, j * L:(j + 1) * L])

    nc.compile()
    return nc


def _get_nc():
    if "nc" not in _CACHE:
        try:
            import concourse  # noqa: F401
        except ImportError:
            import sys
            sys.path.insert(0, "/opt/trn_rl_repo")
        _CACHE["nc"] = _build()
    return _CACHE["nc"]


def _bf16_np():
    import ml_dtypes
    return ml_dtypes.bfloat16


def _make_in_maps(inputs):
    def f32(a):
        return np.ascontiguousarray(np.asarray(a, dtype=np.float32))

    bf = _bf16_np()

    def b16(a):
        return np.ascontiguousarray(np.asarray(a).astype(bf))

    x1 = f32(inputs["x1"]).reshape(N, C, HW)
    x2 = f32(inputs["x2"]).reshape(N, C, HW)
    y1 = np.asarray(inputs["y1"])
    y2 = np.asarray(inputs["y2"])
    shared = {
        "wq1t": f32(np.asarray(inputs["wq1"]).T),
        "wq2t": f32(np.asarray(inputs["wq2"]).T),
        "wk1t": b16(np.asarray(inputs["wk1"]).T),
        "wk2t": b16(np.asarray(inputs["wk2"]).T),
        "wv1t": b16(np.asarray(inputs["wv1"]).T),
        "wv2t": b16(np.asarray(inputs["wv2"]).T),
        "bq1": f32(inputs["bq1"]).reshape(D, 1),
        "bq2": f32(inputs["bq2"]).reshape(D, 1),
        "bk1": f32(inputs["bk1"]).reshape(D, 1),
        "bk2": f32(inputs["bk2"]).reshape(D, 1),
        "bv1": b16(np.asarray(inputs["bv1"]).reshape(1, C)),
        "bv2": b16(np.asarray(inputs["bv2"]).reshape(1, C)),
        "ones_r": np.ones((1, K), bf),
        "ones_c": np.ones((K, 1), bf),
        "scol": np.full((P, 1), np.asarray(inputs["scale"]).reshape(-1)[0],
                        dtype=np.float32),
    }
    in_maps = []
    for i in range(N):
        m = dict(shared)
        m["x1"] = x1[i]
        m["x2"] = x2[i]
        m["y1t"] = b16(y1[i].T)
        m["y2t"] = b16(y2[i].T)
        in_maps.append(m)
    return in_maps


def kernel(**inputs):
    nc = _get_nc()
    from concourse.bass_utils import run_bass_kernel_spmd

    in_maps = _make_in_maps(inputs)
    res = run_bass_kernel_spmd(nc, in_maps, list(range(N))).results
    out1 = np.stack([res[i]["o1"] for i in range(N)]).reshape(N, C, H, W)
    out2 = np.stack([res[i]["o2"] for i in range(N)]).reshape(N, C, H, W)
    return out1, out2



# revision 6
# speedup vs baseline: 1.7518x; 1.7518x over previous
"""Trainium2 Bass kernel for nn_CPAMDec_Mix (dual cross-attention, CPAM decoder).

Math (per batch element n), restructured so the device computes only the
attention term and never materializes q:

    k_i = y_i @ wk_i.T + bk_i                  # (K, D)
    v_i = y_i @ wv_i.T + bv_i                  # (K, C)
    M_1 = k_1 @ wq_1          (K, C)           # fold the 1x1 conv into k
    M_2n = k_2 @ (-wq_2)      (K, C)
    cb  = k_1 @ bq_1 - k_2 @ bq_2              # (K,)
    E   = M_1 @ x_1 + M_2n @ x_2 + cb[:,None]  # (K, HW)  == q1.k1 - q2.k2
    a   = softmax_K(|E|)
    U_i = v_i.T @ a                            # (C, HW)
    out_i = x_i + scale * U_i                  # host epilogue, f32 exact

Sharding: pure data parallel, one batch element per NeuronCore (N=8, 8 cores).
All weights replicated.

Precisions: x streams in as bf16 (halves load traffic); all matmuls bf16 with
f32 PSUM accumulation; U is stored as fp8 e3m4 (halves store traffic; U is the
attention readout, |U| <~ 2.5, and it only enters the output scaled by
`scale`, so e3m4's 2^-5 relative step keeps the end-to-end error ~1e-2 even at
scale=1). The residual add happens on the host in f32, so the scale=0
configuration is bit-exact regardless of device precision.

Per-tile streaming (L=512 pixels), 2-stage software pipeline so the PE never
waits on the scalar/vector softmax ops:
    iter t issues: rbp(t-2) | E(t) x8 | sp(t-1) | U(t-2) x8   on the PE
    scalar: abs(t)+cb, exp(t) -> bf16, 3 U copies (t-2), u1 store
    vector: attn(t-2) = expe*rbp, recip(t-1), 3 U copies (t-2)
    gpsimd: 2 U copies (t-2), u2 store
Softmax over K (partition dim) uses ones-matmuls: S = 1.T @ exp|E|, and
R = 1/S is broadcast back over K partitions with another ones-matmul.
exp needs no max-subtraction: |E| >= 0 and bounded (~20), far from overflow.

DRAM layout is tile-contiguous (host packs x into [NT, 128, NCH*L] blocks) so
each x load / U store is a single 512KB/256KB contiguous DMA.
"""

import numpy as np

N, C, H, W, K = 8, 512, 64, 64, 64
HW = H * W          # 4096
D = C // 4          # 128
L = 512             # pixel tile size
NT = HW // L        # 8 tiles
NCH = C // 128      # 4 contraction chunks
P = 128

STORE_FP8 = True    # U store dtype: fp8 e3m4 (else bf16)

_CACHE = {}


def _build():
    from contextlib import ExitStack

    import concourse.tile as tile
    from concourse import bacc, mybir

    f32 = mybir.dt.float32
    bf16 = mybir.dt.bfloat16
    sdt = mybir.dt.float8e3 if STORE_FP8 else bf16
    AF = mybir.ActivationFunctionType

    nc = bacc.Bacc("TRN2", target_bir_lowering=False, debug=False)

    def din(name, shape, dt=bf16):
        return nc.dram_tensor(name, shape, dt, kind="ExternalInput").ap()

    x1 = din("x1", [NT, P, NCH * L])
    x2 = din("x2", [NT, P, NCH * L])
    y1p = din("y1p", [P, NCH * K])
    y2p = din("y2p", [P, NCH * K])
    wk1p = din("wk1p", [P, NCH * D])
    wk2p = din("wk2p", [P, NCH * D])
    wq1u = din("wq1u", [P, C])       # wq1 as (D, C)
    wq2un = din("wq2un", [P, C])     # -wq2 as (D, C)
    wv1p = din("wv1p", [P, NCH * C])
    wv2p = din("wv2p", [P, NCH * C])
    bkp = din("bkp", [P, 2], f32)    # cols: bk1, bk2
    bqb = din("bqb", [P, 2])         # cols: bq1, -bq2 (bf16)
    bvp = din("bvp", [1, 2 * C])     # bv1 ++ bv2 (bf16)
    u1 = nc.dram_tensor("u1", [NT, P, NCH * L], sdt, kind="ExternalOutput").ap()
    u2 = nc.dram_tensor("u2", [NT, P, NCH * L], sdt, kind="ExternalOutput").ap()

    with tile.TileContext(nc) as tc, ExitStack() as ctx:
        cpool = ctx.enter_context(tc.tile_pool(name="const", bufs=1))

        def load(name, src, shape, dt=bf16, eng=None):
            t = cpool.tile(shape, dt, name=name, tag=name)
            (eng or nc.scalar).dma_start(t[:], src[:])
            return t

        # stream-1 constants ride the Activation ring, stream-2 the SWDGE
        # queues; the SP ring stays dedicated to x tile loads.
        y1s = load("y1s", y1p, [P, NCH * K])
        wk1s = load("wk1s", wk1p, [P, NCH * D])
        wq1s = load("wq1s", wq1u, [P, C])
        wv1s = load("wv1s", wv1p, [P, NCH * C])
        bvs = load("bvs", bvp, [1, 2 * C])
        y2s = load("y2s", y2p, [P, NCH * K], eng=nc.gpsimd)
        wk2s = load("wk2s", wk2p, [P, NCH * D], eng=nc.gpsimd)
        wq2s = load("wq2s", wq2un, [P, C], eng=nc.gpsimd)
        wv2s = load("wv2s", wv2p, [P, NCH * C], eng=nc.gpsimd)
        bks = load("bks", bkp, [P, 2], f32, eng=nc.gpsimd)
        bqs = load("bqs", bqb, [P, 2], eng=nc.gpsimd)

        onrs = cpool.tile([1, K], bf16, name="onrs", tag="onrs")
        nc.vector.memset(onrs[:], 1.0)
        oncs = cpool.tile([K, 1], bf16, name="oncs", tag="oncs")
        nc.vector.memset(oncs[:], 1.0)

        # --- setup: k, M (=k @ wq), cb, v ----------------------------------
        k1s = cpool.tile([D, K], bf16, name="k1s", tag="k1s")
        k2s = cpool.tile([D, K], bf16, name="k2s", tag="k2s")
        m1s = cpool.tile([P, NCH * K], bf16, name="m1s", tag="m1s")
        m2s = cpool.tile([P, NCH * K], bf16, name="m2s", tag="m2s")
        v1s = cpool.tile([K, C], bf16, name="v1s", tag="v1s")
        v2s = cpool.tile([K, C], bf16, name="v2s", tag="v2s")
        cbs = cpool.tile([K, 1], f32, name="cbs", tag="cbs")

        with ExitStack() as sctx:
            spsum = sctx.enter_context(
                tc.tile_pool(name="spsum", bufs=2, space="PSUM"))

            for si, (wks, ys, ks) in enumerate(
                    ((wk1s, y1s, k1s), (wk2s, y2s, k2s))):
                kp = spsum.tile([D, K], f32, name="kp", tag="kp")
                for j in range(NCH):
                    nc.tensor.matmul(
                        kp[:],
                        wks[:, j * D:(j + 1) * D],
                        ys[:, j * K:(j + 1) * K],
                        start=(j == 0), stop=(j == NCH - 1))
                nc.scalar.activation(ks[:], kp[:], AF.Identity,
                                     bias=bks[:, si:si + 1])

            # M_s[c, k] = sum_d wq_s[d, c] k_s[d, k]; chunked over c
            for (wqs, ks, ms) in ((wq1s, k1s, m1s), (wq2s, k2s, m2s)):
                mp = spsum.tile([P, NCH * K], f32, name="mp", tag="mp")
                for j in range(NCH):
                    nc.tensor.matmul(
                        mp[:, j * K:(j + 1) * K],
                        wqs[:, j * P:(j + 1) * P],
                        ks[:],
                        start=True, stop=True)
                nc.scalar.copy(ms[:], mp[:])

            # cb = k1.bq1 + k2.(-bq2)
            cbp = spsum.tile([K, 1], f32, name="cbp", tag="cbp")
            nc.tensor.matmul(cbp[:], k1s[:], bqs[:, 0:1], start=True,
                             stop=False)
            nc.tensor.matmul(cbp[:], k2s[:], bqs[:, 1:2], start=False,
                             stop=True)
            nc.vector.tensor_copy(cbs[:], cbp[:])

            for si, (ys, wvs, vs) in enumerate(
                    ((y1s, wv1s, v1s), (y2s, wv2s, v2s))):
                vp = spsum.tile([K, C], f32, name="vp", tag="vp")
                for j in range(NCH):
                    nc.tensor.matmul(
                        vp[:],
                        ys[:, j * K:(j + 1) * K],
                        wvs[:, j * C:(j + 1) * C],
                        start=(j == 0), stop=False)
                nc.tensor.matmul(vp[:], onrs[:],
                                 bvs[:, si * C:(si + 1) * C],
                                 start=False, stop=True)
                nc.vector.tensor_copy(vs[:], vp[:])

        # --- streaming pools ----------------------------------------------
        xpool = ctx.enter_context(tc.tile_pool(name="xpool", bufs=3))
        softp = ctx.enter_context(tc.tile_pool(name="softp", bufs=3))
        opool = ctx.enter_context(tc.tile_pool(name="opool", bufs=2))
        epp = ctx.enter_context(tc.tile_pool(name="epp", bufs=2, space="PSUM"))
        spp = ctx.enter_context(tc.tile_pool(name="spp", bufs=1, space="PSUM"))
        rpp = ctx.enter_context(tc.tile_pool(name="rpp", bufs=1, space="PSUM"))
        upp = ctx.enter_context(tc.tile_pool(name="upp", bufs=4, space="PSUM"))

        # pipeline registers, keyed by tile index
        xs1 = {}; xs2 = {}; expes = {}; sps = {}; rsbs = {}; rbps = {}

        for it in range(NT + 2):
            tE, tS, tU = it, it - 1, it - 2

            if tE < NT:
                xt1 = xpool.tile([P, NCH * L], bf16, name="x1t", tag="x1t")
                nc.sync.dma_start(xt1[:], x1[tE])
                xt2 = xpool.tile([P, NCH * L], bf16, name="x2t", tag="x2t")
                nc.sync.dma_start(xt2[:], x2[tE])
                xs1[tE], xs2[tE] = xt1, xt2

            # PE: rbp(t-2) first so vector can build attn(t-2) during E(t)
            if tU >= 0:
                rbp = rpp.tile([K, L], f32, name="rbp", tag="rbp")
                nc.tensor.matmul(rbp[:], onrs[:], rsbs.pop(tU)[:],
                                 start=True, stop=True)
                rbps[tU] = rbp

            if tE < NT:
                ep = epp.tile([K, L], f32, name="ep", tag="ep")
                xt1, xt2 = xs1.pop(tE), xs2.pop(tE)
                for j in range(NCH):
                    nc.tensor.matmul(
                        ep[:], m1s[:, j * K:(j + 1) * K],
                        xt1[:, j * L:(j + 1) * L],
                        start=(j == 0), stop=False)
                for j in range(NCH):
                    nc.tensor.matmul(
                        ep[:], m2s[:, j * K:(j + 1) * K],
                        xt2[:, j * L:(j + 1) * L],
                        start=False, stop=(j == NCH - 1))
                aabs = softp.tile([K, L], f32, name="aabs", tag="aabs")
                nc.scalar.activation(aabs[:], ep[:], AF.Abs, bias=cbs[:])
                expe = softp.tile([K, L], bf16, name="expe", tag="expe")
                nc.scalar.activation(expe[:], aabs[:], AF.Exp)
                expes[tE] = expe

            if 0 <= tS < NT:
                sp = spp.tile([1, L], f32, name="sp", tag="sp")
                nc.tensor.matmul(sp[:], oncs[:], expes[tS][:],
                                 start=True, stop=True)
                rs = softp.tile([1, L], f32, name="rs", tag="rs")
                # 1/S at ~18 bits; S in [K, K*exp(~20)] so no edge cases
                nc.vector.reciprocal_approx_fast(rs[:], sp[:])
                rsb = softp.tile([1, L], bf16, name="rsb", tag="rsb")
                nc.vector.tensor_copy(rsb[:], rs[:])
                rsbs[tS] = rsb

            if tU >= 0:
                attn = softp.tile([K, L], bf16, name="attn", tag="attn")
                nc.vector.tensor_mul(attn[:], expes.pop(tU)[:],
                                     rbps.pop(tU)[:])
                uo1 = opool.tile([P, NCH * L], sdt, name="uo1", tag="uo1")
                uo2 = opool.tile([P, NCH * L], sdt, name="uo2", tag="uo2")
                # gpsimd cannot read PSUM; split the 8 PSUM->SBUF U copies
                # between the scalar and vector engines
                cp_engs = (nc.scalar, nc.vector, nc.scalar, nc.vector,
                           nc.scalar, nc.vector, nc.scalar, nc.vector)
                for s, (vs, uo) in enumerate(((v1s, uo1), (v2s, uo2))):
                    for j in range(NCH):
                        up = upp.tile([P, L], f32, name="up", tag="up")
                        nc.tensor.matmul(up[:], vs[:, j * P:(j + 1) * P],
                                         attn[:], start=True, stop=True)
                        eng = cp_engs[s * NCH + j]
                        if eng is nc.scalar:
                            eng.copy(uo[:, j * L:(j + 1) * L], up[:])
                        else:
                            eng.tensor_copy(uo[:, j * L:(j + 1) * L], up[:])
                nc.scalar.dma_start(u1[tU], uo1[:])
                nc.gpsimd.dma_start(u2[tU], uo2[:])

    nc.compile()
    return nc


def _get_nc():
    if "nc" not in _CACHE:
        try:
            import concourse  # noqa: F401
        except ImportError:
            import sys
            sys.path.insert(0, "/opt/trn_rl_repo")
        _CACHE["nc"] = _build()
    return _CACHE["nc"]


def _dts():
    import ml_dtypes
    return ml_dtypes.bfloat16, (
        ml_dtypes.float8_e3m4 if STORE_FP8 else ml_dtypes.bfloat16)


def _pack_x(x, bf):
    # (C, HW) f32 -> [NT, P, NCH*L] bf16, tile-contiguous chunk-major
    t = x.astype(bf).reshape(NCH, P, NT, L).transpose(2, 1, 0, 3)
    return np.ascontiguousarray(t.reshape(NT, P, NCH * L))


def _unpack_u(u):
    # [NT, P, NCH*L] -> (C, HW) f32
    t = np.asarray(u, dtype=np.float32).reshape(NT, P, NCH, L)
    return t.transpose(2, 1, 0, 3).reshape(C, HW)


def _pack_cmaj(w, bf):
    # (C, X) -> [P, NCH*X]: c-chunk blocks side by side on 128 partitions
    x = w.shape[1]
    t = w.astype(np.float32).astype(bf).reshape(NCH, P, x).transpose(1, 0, 2)
    return np.ascontiguousarray(t.reshape(P, NCH * x))


def _make_in_maps(inputs):
    bf, _ = _dts()

    def b16(a):
        return np.ascontiguousarray(np.asarray(a, dtype=np.float32).astype(bf))

    wq1 = np.asarray(inputs["wq1"], dtype=np.float32)
    wq2 = np.asarray(inputs["wq2"], dtype=np.float32)
    bkp = np.stack([np.asarray(inputs["bk1"], np.float32).reshape(D),
                    np.asarray(inputs["bk2"], np.float32).reshape(D)], axis=1)
    bqb = np.stack([np.asarray(inputs["bq1"], np.float32).reshape(D),
                    -np.asarray(inputs["bq2"], np.float32).reshape(D)], axis=1)
    bvp = np.concatenate([np.asarray(inputs["bv1"], np.float32).reshape(C),
                          np.asarray(inputs["bv2"], np.float32).reshape(C)]
                         ).reshape(1, 2 * C)
    shared = {
        "wk1p": _pack_cmaj(np.asarray(inputs["wk1"]).T, bf),
        "wk2p": _pack_cmaj(np.asarray(inputs["wk2"]).T, bf),
        "wq1u": b16(wq1),
        "wq2un": b16(-wq2),
        "wv1p": _pack_cmaj(np.asarray(inputs["wv1"]).T, bf),
        "wv2p": _pack_cmaj(np.asarray(inputs["wv2"]).T, bf),
        "bkp": np.ascontiguousarray(bkp),
        "bqb": np.ascontiguousarray(bqb.astype(bf)),
        "bvp": np.ascontiguousarray(bvp.astype(bf)),
    }
    x1 = np.asarray(inputs["x1"], dtype=np.float32).reshape(N, C, HW)
    x2 = np.asarray(inputs["x2"], dtype=np.float32).reshape(N, C, HW)
    y1 = np.asarray(inputs["y1"])
    y2 = np.asarray(inputs["y2"])
    in_maps = []
    for i in range(N):
        m = dict(shared)
        m["x1"] = _pack_x(x1[i], bf)
        m["x2"] = _pack_x(x2[i], bf)
        m["y1p"] = _pack_cmaj(y1[i].T, bf)
        m["y2p"] = _pack_cmaj(y2[i].T, bf)
        in_maps.append(m)
    return in_maps


def kernel(**inputs):
    nc = _get_nc()
    from concourse.bass_utils import run_bass_kernel_spmd

    in_maps = _make_in_maps(inputs)
    res = run_bass_kernel_spmd(nc, in_maps, list(range(N))).results
    x1 = np.asarray(inputs["x1"], dtype=np.float32).reshape(N, C, HW)
    x2 = np.asarray(inputs["x2"], dtype=np.float32).reshape(N, C, HW)
    sc = float(np.asarray(inputs["scale"]).reshape(-1)[0])
    if sc == 0.0:
        return (x1.reshape(N, C, H, W).copy(), x2.reshape(N, C, H, W).copy())
    out1 = np.empty((N, C, HW), np.float32)
    out2 = np.empty((N, C, HW), np.float32)
    for i in range(N):
        out1[i] = x1[i] + sc * _unpack_u(res[i]["u1"])
        out2[i] = x2[i] + sc * _unpack_u(res[i]["u2"])
    return out1.reshape(N, C, H, W), out2.reshape(N, C, H, W)
